# revision 42
# baseline (speedup 1.0000x reference)
"""Distributed kNN OOD-score kernel for 8 Trainium2 NeuronCores (v5).

Problem: for each of 4*32*32 query vectors (D=768), find the 3 nearest
database vectors (N=20000, squared-L2), average the 3 distances, and
bilinearly upsample the resulting [4,32,32] map to [4,1,512,512].

Sharding (v5, pair-split database): cores 2b and 2b+1 both work on batch
image b. The DATABASE is split between them (core half=0 streams entries
0..9999, half=1 streams 10000..19999 -- 7.7MB fp8 per core instead of
15.4MB replicated, which removes the DMA-starvation the v4 kernel hit),
and each core scores its half against ALL 1024 queries of the batch
(8 query tiles of 128). Per-tile top-3 candidates are then exchanged
within the pair by ONE small AllGather and merged (top-8 over own-8 +
masked gathered-6) to exact top-3 over the full database -- each side's
top-3 is sufficient for the union's top-3, so the payload stays tiny.

Tile order per core: [P_bnd, P_a, P_b, P_c, M_bnd, M_a, M_b, M_c] where
P_* are the partner's map-row blocks and M_* this core's own. The gather
payload is positions 0-4 (everything the partner needs: its four blocks
+ my boundary block for the bilinear halo), so the collective fires at
~50us and its ~22us latency (15us rendezvous + DRAM staging each way)
lands just as the scoring stream drains. The halo ood block is computed
redundantly on both cores from the gathered candidates, so no second
exchange is needed. DMA instructions are kept scarce (the SP sequencer
holds ~650ns per dma_start) and ordered by first use; the DoubleRow
identity rides inside the q8 tensor to save a transfer.

Scoring per 1000-col unit (same fp8 DoubleRow scheme as v4, which is at
the PE fp8 peak): db column pairs are host-folded into a=(x0+x1)/2,
b=(x0-x1)/2 streams; 6 DR matmuls give u,v banks; ScalarE takes |v| to
SBUF fp8; one DR identity-matmul adds it back (u+|v| = max(t0,t1)
exactly); DVE max8 scans the 500 folded maxima. The cross products keep
764 of 768 dims; 4 freed contraction rows carry -||x||^2/2 as a 4-level
fp8 split (x4 stationary scale).

Epilogue: per-tile merges -> mean top-3 distance -> PE-transpose into
map layout -> one [32,20]x[32,512] interpolation matmul (p1) -> two
[20,128]x[20,512] matmuls (p2) -> 2x[128,512] output DMAs.
"""

import sys

if "/opt/trn_rl_repo" not in sys.path:
    sys.path.insert(0, "/opt/trn_rl_repo")

import numpy as np
import ml_dtypes

import concourse.bass as bass
import concourse.bacc as bacc
import concourse.mybir as mybir
import concourse.tile as tile
from concourse import bass_utils

# Problem shape (hardcoded per contract).
B, D, H, W = 4, 768, 32, 32
N = 20000
K_NN = 3
OUT_H = OUT_W = 512
N_CORES = 8

NHALF = N // 2        # db entries per core
N_SC = NHALF // 1000  # 10 units of 1000 cols per query tile
NT = 8                # query tiles per core (128 queries each)
QTOT = NT * 128       # 1024 queries scored per core
QW = QTOT + 128       # q8 width: +128 cols carrying the DR identity
QCOL = [0] + [128 * (p + 1) for p in range(1, NT)]  # q8 col of tile p
NKP = 3               # K pairs: 764 data dims + 4 xh rows = 3 * (2*128)
DX = 764              # cross-term dims (768 minus 4 freed for xh rows)
NCOL = 20             # ood columns entering the upsample (16 own + 4 halo)
XS = 4.0              # xh scale, baked into constant query rows
NEG = -1.0e9          # mask value killing own-rank gather blocks

F32 = mybir.dt.float32
BF16 = mybir.dt.bfloat16
FP8 = mybir.dt.float8e4
AX = mybir.AxisListType
AF = mybir.ActivationFunctionType
ALU = mybir.AluOpType
DR = mybir.MatmulPerfMode.DoubleRow

# Map-row start of each 4-row block, per half, in position order
# [P_bnd, P_a, P_b, P_c, M_bnd, M_a, M_b, M_c].
POS_ROWS = (
    [16, 20, 24, 28, 12, 8, 4, 0],   # half 0 (top, own rows 0-15)
    [12, 8, 4, 0, 16, 20, 24, 28],   # half 1 (bottom, own rows 16-31)
)

# Unit schedule: sc-major over payload positions 0-4 first (their last
# chunk-9 units run right after chunk 9 lands at ~25us, so the payload is
# complete at ~39us and the collective hides), then own tiles 5-7.
STRIPS = (
    [(sc, pos) for sc in range(N_SC) for pos in range(5)]
    + [(sc, pos) for pos in range(5, NT) for sc in range(N_SC)]
)
# merge index m: 0=M_bnd, 1=M_a, 2=M_b, 3=M_c, 4=halo(P_bnd)
# q2 column of the tile each merge scores
MERGE_Q2COL = [4, 5, 6, 7, 0]
DEBUG = False


def _build_program():
    nc = bacc.Bacc(
        "TRN2", target_bir_lowering=False, debug=False, num_devices=N_CORES
    )
    q8d = nc.dram_tensor("q8", [128, NKP, 2, QW], FP8, kind="ExternalInput").ap()
    db8d = nc.dram_tensor(
        "db8", [128, NKP, 2, NHALF], FP8, kind="ExternalInput"
    ).ap()
    q2d = nc.dram_tensor("q2", [128, 5, K_NN], F32, kind="ExternalInput").ap()
    s4d = nc.dram_tensor("s4", [128, 4, W], F32, kind="ExternalInput").ap()
    maskd = nc.dram_tensor("mask", [128, 5, 2, K_NN], F32, kind="ExternalInput").ap()
    artd = nc.dram_tensor("art", [NCOL, 2, 128], BF16, kind="ExternalInput").ap()
    acd = nc.dram_tensor("ac", [W, OUT_W], BF16, kind="ExternalInput").ap()
    out = nc.dram_tensor("out", [2, 128, OUT_W], BF16, kind="ExternalOutput").ap()
    if DEBUG:
        dbg_loc8 = nc.dram_tensor(
            "dbg_loc8", [128, NT, 8], F32, kind="ExternalOutput"
        ).ap()
        dbg_oodht = nc.dram_tensor(
            "dbg_oodht", [W, NCOL], BF16, kind="ExternalOutput"
        ).ap()

    with tile.TileContext(nc) as tc:
        with (
            tc.tile_pool(name="static", bufs=1) as sp,
            tc.tile_pool(name="db", bufs=N_SC * NKP) as dbp,
            tc.tile_pool(name="absv", bufs=5) as avp,
            tc.tile_pool(name="small", bufs=8) as smp,
            tc.tile_pool(name="psum", bufs=4, space="PSUM") as pp,
            tc.tile_pool(name="dram", bufs=1, space="DRAM") as dp,
        ):
            # ---- input DMAs, in first-need order. The SP sequencer holds
            # ~650ns per dma_start, so the count is kept low: 3 q8 pieces
            # (the DR identity rides in q8 cols 128:256), per-kp chunks for
            # sc0 only, whole-chunk DMAs for sc1-9, misc last. ----
            q8 = sp.tile([128, NKP, 2, QW], FP8)
            # pos0 queries first (first matmul), identity block second
            # (first drain, ~3 units later)
            nc.sync.dma_start(q8[:, :, :, 0:128], q8d[:, :, :, 0:128])
            dbt = {}  # sc -> [128, NKP, 2, 1000] tile
            db0 = dbp.tile([128, NKP, 2, 1000], FP8, tag="db", name="db0")
            db1 = dbp.tile([128, NKP, 2, 1000], FP8, tag="db", name="db1")
            # sc0 and sc1 stream per K-pair, query pieces per tile, each
            # landing just before first use -- the first ~10 units are
            # DMA-throughput-bound, so granularity is everything here
            nc.sync.dma_start(db0[:, 0, :, :], db8d[:, 0, :, 0:1000])
            nc.sync.dma_start(q8[:, :, :, 256:384], q8d[:, :, :, 256:384])
            nc.sync.dma_start(db0[:, 1, :, :], db8d[:, 1, :, 0:1000])
            nc.sync.dma_start(q8[:, :, :, 384:512], q8d[:, :, :, 384:512])
            nc.sync.dma_start(db0[:, 2, :, :], db8d[:, 2, :, 0:1000])
            dbt[0] = db0
            nc.sync.dma_start(q8[:, :, :, 128:256], q8d[:, :, :, 128:256])
            nc.sync.dma_start(q8[:, :, :, 512:640], q8d[:, :, :, 512:640])
            nc.sync.dma_start(db1[:, 0, :, :], db8d[:, 0, :, 1000:2000])
            nc.sync.dma_start(q8[:, :, :, 640:768], q8d[:, :, :, 640:768])
            nc.sync.dma_start(db1[:, 1, :, :], db8d[:, 1, :, 1000:2000])
            nc.sync.dma_start(db1[:, 2, :, :], db8d[:, 2, :, 1000:2000])
            dbt[1] = db1
            for sc in range(2, N_SC):
                t = dbp.tile([128, NKP, 2, 1000], FP8, tag="db", name=f"db{sc}")
                nc.sync.dma_start(
                    t[:], db8d[:, :, :, sc * 1000 : (sc + 1) * 1000]
                )
                dbt[sc] = t
                if sc == 4:
                    nc.sync.dma_start(q8[:, :, :, 768:QW], q8d[:, :, :, 768:QW])
            q2_sb = sp.tile([128, 5, K_NN], F32)
            nc.sync.dma_start(q2_sb[:], q2d[:])
            ident = q8[:, 0, :, 128:256]
            mask_sb = sp.tile([128, 5, 2, K_NN], F32)
            nc.sync.dma_start(mask_sb[:], maskd[:])
            s4 = sp.tile([128, 4, W], F32)
            nc.sync.dma_start(s4[:], s4d[:])
            art_sb = sp.tile([NCOL, 2, 128], BF16)
            nc.sync.dma_start(art_sb[:], artd[:])
            ac_sb = sp.tile([W, OUT_W], BF16)
            nc.sync.dma_start(ac_sb[:], acd[:])

            # ---- working state ----
            parts = [
                sp.tile([128, (N_SC + 1) * 8], F32, name=f"part{p}")
                for p in range(NT)
            ]
            # payload top-8s, k-major so the staged top-3 block is a
            # single contiguous run per partition (128 SWDGE descs)
            loc8_pay = sp.tile([128, 8, 5], F32)
            # merge staging: slot0 = own top-8, slots 1-2 = masked gather
            mrg_all = sp.tile([128, 5, 3, 8], F32)
            cc_in = dp.tile([128 * 5 * K_NN], F32)
            cc_out = dp.tile([2 * 128 * 5 * K_NN], F32)
            rem8 = sp.tile([128, 2, 5 * K_NN], F32)    # gathered payloads
            ood_hT = sp.tile([W, NCOL], BF16)
            nc.gpsimd.memset(mrg_all[:, :, 1:3, K_NN:8], NEG)

            def rhs(sc, kp, col, width):
                return dbt[sc][:, kp, :, col : col + width]

            pending = []

            def drain_one():
                u, absv, sw, part_ap = pending.pop(0)
                nc.tensor.matmul(
                    u[:, 0, 0:sw],
                    ident,
                    absv[:, :, 0:sw],
                    start=False,
                    stop=True,
                    perf_mode=DR,
                )
                nc.vector.max(part_ap, u[:, 0, 0:sw])

            def tile_epilogue(pos):
                m = pos - 4  # merge index for own tiles (pos 5,6,7 -> 1,2,3)
                if pos <= 4:
                    out8 = loc8_pay[:, :, pos]
                else:
                    out8 = mrg_all[:, m, 0, :]
                nw = (N_SC + 1) * 8 if pos == NT - 1 else N_SC * 8
                nc.vector.max(out8, parts[pos][:, 0:nw])
                if pos == 4:
                    # payload complete: pair-exchange positions 0-4
                    # (gpsimd-issued DMAs: 25ns sequencer hold vs 650 on SP)
                    nc.gpsimd.dma_start(cc_in[:], loc8_pay[:, 0:K_NN, :])
                    nc.gpsimd.collective_compute(
                        "AllGather",
                        ALU.bypass,
                        replica_groups=[[0, 1], [2, 3], [4, 5], [6, 7]],
                        ins=[cc_in.opt()],
                        outs=[cc_out.opt()],
                    )
                    nc.sync.dma_start(
                        rem8[:],
                        cc_out.rearrange("(r p f) -> p r f", r=2, p=128),
                    )

            # last strip index of each tile, and where to emit its epilogue
            # (>=3 units later so the pending drain has naturally passed it)
            last_idx = {}
            for i, (sc, pos) in enumerate(STRIPS):
                last_idx[pos] = i
            emit_at = {}
            for pos, idx in last_idx.items():
                emit_at.setdefault(min(idx + 2, len(STRIPS) - 1), []).append(pos)

            memset_count = 0
            for si, (sc, pos) in enumerate(STRIPS):
                c0 = QCOL[pos]
                lhsT = [
                    q8[:, kp, :, c0 : c0 + 128] for kp in range(NKP)
                ]
                last_split = si == len(STRIPS) - 1
                subs = [(0, 250), (250, 250)] if last_split else [(0, 500)]
                for hs, (s0, sw) in enumerate(subs):
                    u_ps = pp.tile([128, 1, 512], F32, tag="u", name="u", bufs=4)
                    v_ps = pp.tile([128, 1, 512], F32, tag="v", name="v", bufs=4)
                    for kp in range(NKP):
                        nc.tensor.matmul(
                            u_ps[:, 0, 0:sw],
                            lhsT[kp],
                            rhs(sc, kp, s0, sw),
                            start=(kp == 0),
                            stop=False,
                            perf_mode=DR,
                        )
                        nc.tensor.matmul(
                            v_ps[:, 0, 0:sw],
                            lhsT[kp],
                            rhs(sc, kp, 500 + s0, sw),
                            start=(kp == 0),
                            stop=(kp == NKP - 1),
                            perf_mode=DR,
                        )
                    absv = avp.tile([128, 2, 500], FP8, tag="absv", name="absv")
                    if memset_count < 5:
                        nc.gpsimd.memset(absv[:, 1, 0:500], 0.0)
                        memset_count += 1
                    nc.scalar.activation(
                        absv[:, 0, 0:sw], v_ps[:, 0, 0:sw], AF.Abs
                    )
                    pending.append(
                        (u_ps, absv, sw,
                         parts[pos][:, (sc + hs) * 8 :][:, 0:8])
                    )
                    while len(pending) > 2:
                        drain_one()
                for pos_done in emit_at.get(si, []):
                    if last_idx[pos_done] < si - 1:
                        tile_epilogue(pos_done)
                    else:
                        # too fresh (end of stream): flush pending first
                        while pending:
                            drain_one()
                        tile_epilogue(pos_done)
            while pending:
                drain_one()

            # own-top8 slots for the bnd and halo merges come from the payload
            nc.vector.tensor_copy(mrg_all[:, 0, 0, :], loc8_pay[:, :, 4])
            nc.vector.tensor_copy(mrg_all[:, 4, 0, :], loc8_pay[:, :, 0])

            # ---- merges: one batched mask-add, then top-8 per tile ----
            nc.vector.tensor_tensor(
                mrg_all[:, :, 1:3, 0:K_NN],
                rem8.rearrange("p r (f j) -> p j r f", j=5),
                mask_sb[:],
                op=ALU.add,
            )
            t8_all = smp.tile([128, 5, 8], F32, tag="t8", name="t8_all")
            for m in range(5):
                nc.vector.max(t8_all[:, m, :], mrg_all[:, m, :, :])
            # x = (q2 - 2 t)/9 for all 5 merges in one DVE op
            x3 = smp.tile([128, 5, K_NN], F32, tag="x3", name="x3")
            nc.vector.scalar_tensor_tensor(
                x3[:], t8_all[:, :, 0:K_NN], -2.0 / 9.0, q2_sb[:],
                op0=ALU.mult, op1=ALU.add,
            )
            # transpose into map layout BEFORE the sqrt: 20 tiny f32
            # matmuls, then ONE 60-wide sqrt and ONE 3-sum -> ood_hT
            xT_ps = pp.tile([W, 5, 4, K_NN], F32, tag="u", name="xT", bufs=4)
            for m in range(5):
                for blk in range(4):
                    nc.tensor.matmul(
                        xT_ps[:, m, blk, :],
                        s4[:, blk, :],
                        x3[:, m, :],
                        start=True,
                        stop=True,
                    )
            d3T = smp.tile([W, NCOL, K_NN], F32, tag="d3", name="d3T")
            nc.scalar.activation(
                d3T[:], xT_ps.rearrange("c m b k -> c (m b) k"), AF.Sqrt
            )
            with nc.allow_low_precision(
                reason="3-element sum rounded to bf16 for the upsample "
                "matmul; ~0.2% on a 2% tolerance"
            ):
                nc.vector.reduce_sum(ood_hT[:], d3T[:], axis=AX.X)

            if DEBUG:
                nc.sync.dma_start(
                    dbg_loc8[:, 0:5, :].rearrange("p a b -> p (a b)"),
                    loc8_pay[:].rearrange("p a b -> p (a b)"),
                )
                nc.sync.dma_start(dbg_oodht[:], ood_hT[:])

            # ---- bilinear upsample ----
            # each big matmul starts with an 8-col warmup slice so the bulk
            # runs at the mid p-state instead of cold; the copy/p2/out
            # chain is column-split so the two halves pipeline
            p1_ps = pp.tile([NCOL, OUT_W], F32, tag="v", name="p1", bufs=4)
            nc.tensor.matmul(
                p1_ps[:, 0:8], ood_hT[:], ac_sb[:, 0:8], start=True, stop=True
            )
            nc.tensor.matmul(
                p1_ps[:, 8:256], ood_hT[:], ac_sb[:, 8:256], start=True, stop=True
            )
            nc.tensor.matmul(
                p1_ps[:, 256:512], ood_hT[:], ac_sb[:, 256:512],
                start=True, stop=True,
            )
            p1_sb = sp.tile([NCOL, OUT_W], BF16)
            p2s, osbs = [], []
            for m2 in range(2):
                p2 = pp.tile([128, OUT_W], F32, tag="v", name=f"p2_{m2}", bufs=4)
                o_sb = smp.tile([128, OUT_W], BF16, tag="osb", name=f"osb{m2}")
                p2s.append(p2)
                osbs.append(o_sb)
            nc.scalar.activation(p1_sb[:, 0:256], p1_ps[:, 0:256], AF.Copy)
            with nc.allow_low_precision(reason="bf16 interp stage"):
                nc.vector.tensor_copy(p1_sb[:, 256:512], p1_ps[:, 256:512])
            nc.tensor.matmul(
                p2s[1][:, 0:8], art_sb[:, 1, :], p1_sb[:, 0:8],
                start=True, stop=True,
            )
            nc.tensor.matmul(
                p2s[1][:, 8:256], art_sb[:, 1, :], p1_sb[:, 8:256],
                start=True, stop=True,
            )
            nc.tensor.matmul(
                p2s[0][:, 0:256], art_sb[:, 0, :], p1_sb[:, 0:256],
                start=True, stop=True,
            )
            nc.tensor.matmul(
                p2s[1][:, 256:512], art_sb[:, 1, :], p1_sb[:, 256:512],
                start=True, stop=True,
            )
            nc.tensor.matmul(
                p2s[0][:, 256:512], art_sb[:, 0, :], p1_sb[:, 256:512],
                start=True, stop=True,
            )
            # block 0 drains via ScalarE copy + SP-issued DMA, block 1 via
            # DVE copy + Pool-issued (SWDGE) DMA -- fully parallel lanes
            nc.scalar.activation(osbs[0][:], p2s[0][:], AF.Copy)
            nc.sync.dma_start(out[0, :, :], osbs[0][:])
            with nc.allow_low_precision(reason="bf16 output map"):
                nc.vector.tensor_copy(osbs[1][:], p2s[1][:])
            nc.gpsimd.dma_start(out[1, :, :], osbs[1][:])

    nc.compile()
    return nc


def _bilinear_matrix(out_size: int, in_size: int) -> np.ndarray:
    """Half-pixel (align_corners=False) bilinear interpolation matrix
    [out_size, in_size]; edge-clamped, equivalent to jax.image.resize
    'bilinear' for integer upsampling."""
    A = np.zeros((out_size, in_size), dtype=np.float64)
    scale = in_size / out_size
    for i in range(out_size):
        s = (i + 0.5) * scale - 0.5
        j0 = int(np.floor(s))
        w = s - j0
        A[i, min(max(j0, 0), in_size - 1)] += 1.0 - w
        A[i, min(max(j0 + 1, 0), in_size - 1)] += w
    return A.astype(np.float32)


_NC_CACHE = None


def _get_nc():
    global _NC_CACHE
    if _NC_CACHE is None:
        _NC_CACHE = _build_program()
    return _NC_CACHE


def _slot_pack(db_half: np.ndarray):
    """Fold all column pairs of one db half [NHALF, 768]: per 1000-col
    group the slot layout is [500 a=(x0+x1)/2 | 500 b=(x0-x1)/2] over 764
    dims, with the matching xh = -(||x0||^2 +- ||x1||^2)/4 terms (over all
    768 dims, scaled 1/XS) as a 4-level fp8 split in rows 764..767."""
    n = db_half.shape[0]
    h = 0.5 * np.einsum("nd,nd->n", db_half, db_half)
    dbX = np.empty((n, 768), dtype=np.float32)
    xhX = np.empty(n, dtype=np.float32)
    for g in range(n // 1000):
        base = g * 1000
        p0 = db_half[base : base + 1000 : 2, :DX]
        p1 = db_half[base + 1 : base + 1000 : 2, :DX]
        h0 = h[base : base + 1000 : 2]
        h1 = h[base + 1 : base + 1000 : 2]
        dbX[base : base + 500, :DX] = 0.5 * (p0 + p1)
        dbX[base + 500 : base + 1000, :DX] = 0.5 * (p0 - p1)
        xhX[base : base + 500] = -0.5 * (h0 + h1)
        xhX[base + 500 : base + 1000] = -0.5 * (h0 - h1)
    r = (xhX / XS).astype(np.float32)
    for lv in range(4):
        q = r.astype(ml_dtypes.float8_e4m3).astype(np.float32)
        dbX[:, DX + lv] = q
        r = r - q
    return dbX


def make_in_maps(embeddings: np.ndarray, database: np.ndarray):
    embeddings = np.asarray(embeddings, dtype=np.float32)
    database = np.asarray(database, dtype=np.float32)

    q_all = embeddings.transpose(0, 2, 3, 1).reshape(B, H * W, D)
    A512 = _bilinear_matrix(OUT_W, W)                    # [512, 32]
    s4 = np.eye(128, dtype=np.float32).reshape(128, 4, W)
    ac = np.ascontiguousarray(A512.T).astype(ml_dtypes.bfloat16)  # [32, 512]

    db8_half = []
    for half in range(2):
        dbX = _slot_pack(database[half * NHALF : (half + 1) * NHALF])
        db8_half.append(
            np.ascontiguousarray(
                dbX.T.reshape(NKP, 2, 128, NHALF).transpose(2, 0, 1, 3)
            ).astype(ml_dtypes.float8_e4m3)
        )

    in_maps = []
    for c in range(N_CORES):
        b, half = divmod(c, 2)
        pos_rows = POS_ROWS[half]

        # queries of all 8 tiles in position order; the DR identity
        # block occupies cols 128:256 (kp0 ktile0 = I, rest 0)
        q = np.concatenate(
            [q_all[b, r0 * W : (r0 + 4) * W] for r0 in pos_rows]
        )                                                # [1024, 768]
        qX = np.zeros((QW, 768), dtype=np.float32)
        qX[0:128] = q[0:128]
        qX[0:128, DX:] = XS
        qX[128:256, 0:128] = np.eye(128, dtype=np.float32)
        qX[256:QW] = q[128:QTOT]
        qX[256:QW, DX:] = XS
        q8 = np.ascontiguousarray(
            qX.T.reshape(NKP, 2, 128, QW).transpose(2, 0, 1, 3)
        ).astype(ml_dtypes.float8_e4m3)                  # [128, 3, 2, 1152]
        q2 = np.einsum("qd,qd->q", q, q) / 9.0
        q2 = q2.reshape(NT, 128).T.astype(np.float32)      # [128, NT]
        q2m = np.ascontiguousarray(
            np.repeat(q2[:, MERGE_Q2COL, None], K_NN, axis=2)
        ).astype(np.float32)                               # [128, 5, 3]

        # gather mask: my own rank's block can never win a merge
        mask = np.zeros((128, 5, 2, K_NN), dtype=np.float32)
        mask[:, :, c % 2, :] = NEG

        # art: j-columns [M_bnd, M_a, M_b, M_c, halo] x 4 rows each
        grow = [pos_rows[4 + t] + r for t in range(4) for r in range(4)] + [
            pos_rows[0] + r for r in range(4)
        ]
        rowsA = A512[half * 256 : (half + 1) * 256]      # [256, 32]
        art = np.zeros((NCOL, 2, 128), dtype=np.float32)
        for j, g in enumerate(grow):
            art[j, 0, :] = rowsA[0:128, g]
            art[j, 1, :] = rowsA[128:256, g]

        in_maps.append(
            {
                "db8": db8_half[half],
                "q8": q8,
                "q2": q2m,
                "s4": s4,
                "mask": mask,
                "art": art.astype(ml_dtypes.bfloat16),
                "ac": ac,
            }
        )
    return in_maps


def run_device(in_maps, **kwargs):
    nc = _get_nc()
    return bass_utils.run_bass_kernel_spmd(
        nc, in_maps, core_ids=list(range(N_CORES)), **kwargs
    )


def kernel(embeddings, database, k, out_h, out_w):
    assert int(k) == K_NN and int(out_h) == OUT_H and int(out_w) == OUT_W
    in_maps = make_in_maps(np.asarray(embeddings), np.asarray(database))
    res = run_device(in_maps)
    out = np.empty((B, 1, OUT_H, OUT_W), dtype=np.float32)
    for c in range(N_CORES):
        b, half = divmod(c, 2)
        o = np.asarray(res.results[c]["out"], dtype=np.float32)
        out[b, 0, half * 256 : half * 256 + 128] = o[0]
        out[b, 0, half * 256 + 128 : (half + 1) * 256] = o[1]
    return out
